# revision 1
# baseline (speedup 1.0000x reference)
"""Low-rank self-attention Trainium2 kernel.

Sharding: batch x sequence-half data parallel across 8 cores.
Core c handles batch b=c//2, query half h=c%2. The host rolls x[b] so the
local query rows come first; softmax/PV sums over k are permutation
invariant, so the result is exact.

Per-core pipeline (Sq=2048 queries, Sk=4096 keys, D=1024, R=32):
  A. x (bf16, host-cast) -> PE-transpose -> x^T ; QKV^T = Wqkv^T @ x^T
     (bf16 MMs, fp32 psum, bias fused on ACT); replicate Q^T/K^T to 4
     partition groups; V natural + ones column (denominator) in bf16.
  B. per 512-query chunk: scores^T = K^T.T @ Q^T (4-way row-packed fp32r,
     rank-32 contraction); expS^T = exp(scale*scores^T) (ACT, bf16);
     attn^T[33, q] accumulated over 32 k-tiles (row 32 = denominator).
  C. denominators PE-transposed to [128q, 16] partition layout; y =
     (attn^T.T @ Wo) * (1/den) + bo with the normalize+bias fused on DVE.
"""
import sys

sys.path.insert(0, "/opt/trn_rl_repo")

import numpy as np
import ml_dtypes

import concourse.bass as bass
import concourse.mybir as mybir
import concourse.tile as tile
from concourse.bass_utils import run_bass_kernel_spmd
from bass_rust import ScopedClock

BF16 = mybir.dt.bfloat16
F32 = mybir.dt.float32
F32R = mybir.dt.float32r

B, S, D, R = 4, 4096, 1024, 32
SQ = S // 2
N_CORES = 8
SCALE = float(R) ** -0.5


class ChunkedDrainTileContext(tile.TileContext):
    """This walrus build rejects >1 sync wait on the kernel-tail drain;
    spread the final drain's waits across single-wait SP nops."""

    def _drain_and_barrier(self, tick_clock, wait_clock):
        nc = self.nc
        MAX_NOPS = 40
        nops = [nc.sync.nop(nofuse=True) for _ in range(MAX_NOPS)]
        drain_inst = nc.sync.drain()
        wait_clock.add_sem_waits(
            drain_inst.ins, ScopedClock({None: tick_clock.global_clock})
        )
        si = drain_inst.ins.sync_info
        waits = list(si.on_wait) if si and si.on_wait else []
        if len(waits) > 1:
            assert len(waits) <= 1 + MAX_NOPS, f"too many drain waits: {len(waits)}"
            drain_inst.ins.sync_info = mybir.SyncInfo(
                on_wait=[waits[0]], on_update=si.on_update
            )
            for i, w in enumerate(waits[1:]):
                nop = nops[i]
                old = nop.ins.sync_info
                nop.ins.sync_info = mybir.SyncInfo(
                    on_wait=[w], on_update=old.on_update if old else []
                )
        nc.all_engine_barrier()
        assert self.sems is not None
        popped = nc._tile_sem_poison_stack.pop()
        assert popped is self._sem_poison
        nc.clear_and_free_semaphores(list(self.sems.allocated().values()))
        nc.all_engine_barrier()
        split_multi_waits(nc)


def split_multi_waits(nc):
    """walrus in this container rejects instructions with more than one sync
    wait; split extras onto same-engine nops placed immediately before."""
    for f in nc.m.functions:
        for bb in f.blocks:
            snap = list(bb.instructions)
            if not any(
                inst.sync_info and inst.sync_info.on_wait
                and len(inst.sync_info.on_wait) > 1
                for inst in snap
            ):
                continue
            newlist = []
            created = set()
            for inst in snap:
                si = inst.sync_info
                waits = list(si.on_wait) if si and si.on_wait else []
                if len(waits) > 1:
                    eng = inst.engine
                    for w in waits[:-1]:
                        nop = nc.engines[eng].nop(nofuse=True)
                        nop.ins.sync_info = mybir.SyncInfo(
                            on_wait=[w], on_update=[]
                        )
                        created.add(nop.ins.name)
                        newlist.append(nop.ins)
                    inst.sync_info = mybir.SyncInfo(
                        on_wait=[waits[-1]], on_update=si.on_update
                    )
                newlist.append(inst)
            # nops were auto-appended to the current bb; strip strays
            for f2 in nc.m.functions:
                for bb2 in f2.blocks:
                    if bb2 is bb:
                        continue
                    cur = list(bb2.instructions)
                    if any(i.name in created for i in cur):
                        bb2.instructions = [
                            i for i in cur if i.name not in created
                        ]
            # also strip auto-appended copies at the end of this bb
            tail = [i for i in bb.instructions if i.name in created
                    and i not in snap]
            seen = set()
            final = []
            for i in newlist:
                if i.name in seen:
                    continue
                seen.add(i.name)
                final.append(i)
            bb.instructions = final


def r32(ap):
    return ap.bitcast(F32R)


def build_kernel():
    nc = bass.Bass("TRN2", target_bir_lowering=False, debug=False)

    xb = nc.dram_tensor("xb", [S, D], BF16, kind="ExternalInput")
    wqkv = nc.dram_tensor("wqkv", [D, 96], BF16, kind="ExternalInput")
    bqkv = nc.dram_tensor("bqkv", [96, 1], F32, kind="ExternalInput")
    wo = nc.dram_tensor("wo", [128, D], F32R, kind="ExternalInput")
    bo_t = nc.dram_tensor("bo_t", [128, D], F32, kind="ExternalInput")
    iden = nc.dram_tensor("iden", [128, 128], BF16, kind="ExternalInput")
    onec = nc.dram_tensor("onec", [128, 32], BF16, kind="ExternalInput")
    onef = nc.dram_tensor("onef", [1, 1], F32, kind="ExternalInput")
    y = nc.dram_tensor("y", [SQ, D], F32, kind="ExternalOutput")

    NT = S // 128
    NQT = SQ // 128
    NKT = S // 128
    NQC = SQ // 512
    Exp = mybir.ActivationFunctionType.Exp
    Ident = mybir.ActivationFunctionType.Identity

    with ChunkedDrainTileContext(nc) as tc:
        with (
            tc.tile_pool(name="persist", bufs=1) as pp,
            tc.tile_pool(name="work", bufs=3) as wp,
            tc.tile_pool(name="expp", bufs=2) as ep,
            tc.tile_pool(name="ps1", bufs=1, space="PSUM") as ps1,
        ):
            iden_sb = pp.tile([128, 128], BF16)
            nc.sync.dma_start(iden_sb[:], iden.ap())
            onec_sb = pp.tile([128, 32], BF16)
            nc.sync.dma_start(onec_sb[:], onec.ap())
            onef_sb = pp.tile([1, 1], F32)
            nc.sync.dma_start(onef_sb[:], onef.ap())
            wqkv_sb = pp.tile([128, 8, 96], BF16)
            nc.sync.dma_start(wqkv_sb[:], wqkv.ap().rearrange("(c p) j -> p c j", p=128))
            bqkv_sb = pp.tile([96, 1], F32)
            nc.sync.dma_start(bqkv_sb[:], bqkv.ap())
            wo_sb = pp.tile([128, D], F32R)
            nc.sync.dma_start(wo_sb[:], wo.ap())
            bo_sb = pp.tile([128, D], F32)
            nc.sync.dma_start(bo_sb[:], bo_t.ap())

            qkvT = pp.tile([96, S], F32R)
            qT_rep = pp.tile([128, SQ], F32R)
            kT_rep = pp.tile([128, S], F32R)
            vone = pp.tile([128, NKT, 33], BF16)
            attnT = pp.tile([32, SQ], F32R)
            den = pp.tile([1, SQ], F32)
            rq = pp.tile([128, NQT], F32)
            vTb = pp.tile([32, S], BF16)

            # ================= phase A =================
            with tc.tile_pool(name="psA", bufs=2, space="PSUM") as psA:
                for sc in range(NT // 4):
                    xT = wp.tile([128, 8, 512], BF16, tag="xT")
                    for dc in range(8):
                        nc.sync.dma_start_transpose(
                            xT[:, dc, :],
                            xb.ap()[sc * 512:(sc + 1) * 512,
                                    dc * 128:(dc + 1) * 128],
                        )
                    pq = psA.tile([96, 512], F32, tag="pq")
                    for dc in range(8):
                        nc.tensor.matmul(
                            pq[:], wqkv_sb[:, dc, :], xT[:, dc, :],
                            start=(dc == 0), stop=(dc == 7),
                        )
                    nc.scalar.activation(
                        qkvT[:, sc * 512:(sc + 1) * 512], pq[:], Ident,
                        bias=bqkv_sb[:],
                    )

                for i in range(4):
                    nc.sync.dma_start(qT_rep[32 * i:32 * i + 32, :], qkvT[0:32, 0:SQ])
                    nc.sync.dma_start(kT_rep[32 * i:32 * i + 32, :], qkvT[32:64, :])

                nc.vector.tensor_copy(out=vTb[:], in_=qkvT[64:96, :])
                vt_ps = ps1.tile([128, NKT, 32], BF16, tag="vt")
                for kt in range(NKT):
                    nc.tensor.matmul(
                        vt_ps[:, kt, :], vTb[:, kt * 128:(kt + 1) * 128],
                        iden_sb[0:32, 0:32], is_transpose=True,
                        skip_group_check=True, tile_position=(0, 0),
                    )
                nc.vector.tensor_copy(out=vone[:, :, 0:32], in_=vt_ps[:])
                nc.vector.tensor_copy(out=vone[:, :, 32], in_=onec_sb[:])

            # ================= phase B =================
            with (
                tc.tile_pool(name="psB", bufs=1, space="PSUM") as psB,
                tc.tile_pool(name="psB2", bufs=2, space="PSUM") as psB2,
            ):
                for qc in range(NQC):
                    expT = ep.tile([128, NKT, 512], BF16, tag="expT")
                    for g in range(NKT // 4):
                        ps_s = psB.tile([128, 4, 512], F32, tag="ps_s")
                        for i in range(4):
                            kt = g * 4 + i
                            nc.tensor.matmul(
                                ps_s[:, i, :],
                                (kT_rep[32 * i:32 * i + 32,
                                           kt * 128:(kt + 1) * 128]),
                                (qT_rep[32 * i:32 * i + 32,
                                           qc * 512:(qc + 1) * 512]),
                                start=True, stop=True,
                                skip_group_check=True,
                                tile_position=(32 * i, 0),
                            )
                        nc.scalar.activation(
                            expT[:, g * 4:(g + 1) * 4, :], ps_s[:], Exp,
                            scale=SCALE,
                        )
                    pa = psB2.tile([128, 512], F32, tag="pa")
                    for kt in range(NKT):
                        nc.tensor.matmul(
                            pa[0:33, :], vone[:, kt, :], expT[:, kt, :],
                            start=(kt == 0), stop=(kt == NKT - 1),
                        )
                    nc.vector.tensor_copy(
                        out=attnT[:, qc * 512:(qc + 1) * 512], in_=pa[0:32, :]
                    )
                    nc.vector.tensor_copy(
                        out=den[:, qc * 512:(qc + 1) * 512], in_=pa[32:33, :]
                    )

            # ================= phase C =================
            with tc.tile_pool(name="psC", bufs=2, space="PSUM") as psC:
                rq_ps = ps1.tile([128, NQT], F32, tag="rqps")
                for qt in range(NQT):
                    nc.tensor.matmul(
                        rq_ps[:, qt:qt + 1], den[:, qt * 128:(qt + 1) * 128],
                        onef_sb[:], is_transpose=True,
                        skip_group_check=True, tile_position=(0, 0),
                    )
                nc.vector.reciprocal(rq[:], rq_ps[:])

                atr = pp.tile([128, SQ], F32R)
                for i in range(4):
                    nc.sync.dma_start(atr[32 * i:32 * i + 32, :], attnT[:])

                for qt in range(NQT):
                    i = qt % 4
                    for dc2 in range(2):
                        py = psC.tile([128, 512], F32, tag="py")
                        nc.tensor.matmul(
                            py[:],
                            (atr[32 * i:32 * i + 32, qt * 128:(qt + 1) * 128]),
                            (wo_sb[32 * i:32 * i + 32,
                                      dc2 * 512:(dc2 + 1) * 512]),
                            start=True, stop=True,
                            tile_position=(32 * i, 0),
                        )
                        yt = wp.tile([128, 512], F32, tag="yt")
                        nc.vector.scalar_tensor_tensor(
                            out=yt[:], in0=py[:], scalar=rq[:, qt:qt + 1],
                            in1=bo_sb[:, dc2 * 512:(dc2 + 1) * 512],
                            op0=mybir.AluOpType.mult, op1=mybir.AluOpType.add,
                        )
                        nc.sync.dma_start(
                            y.ap()[qt * 128:(qt + 1) * 128,
                                   dc2 * 512:(dc2 + 1) * 512],
                            yt[:],
                        )
    return nc


_CACHE = {}


def _get_nc():
    if "nc" not in _CACHE:
        _CACHE["nc"] = build_kernel()
    return _CACHE["nc"]


def make_in_maps(x, Wq, bq, Wk, bk, Wv, bv, Wo, bo):
    wqkv = np.concatenate([Wq, Wk, Wv], axis=1).astype(ml_dtypes.bfloat16)
    bqkv = np.concatenate([bq, bk, bv])[:, None].astype(np.float32)
    wo_rep = np.tile(Wo, (4, 1)).astype(np.float32)
    bo_t = np.tile(bo[None, :], (128, 1)).astype(np.float32)
    iden = np.eye(128, dtype=ml_dtypes.bfloat16)
    onec = np.ones((128, 32), dtype=ml_dtypes.bfloat16)
    onef = np.ones((1, 1), np.float32)
    in_maps = []
    for c in range(N_CORES):
        b, h = c // 2, c % 2
        xb_roll = np.roll(x[b], -h * SQ, axis=0).astype(ml_dtypes.bfloat16)
        in_maps.append({
            "xb": xb_roll, "wqkv": wqkv, "bqkv": bqkv, "wo": wo_rep,
            "bo_t": bo_t, "iden": iden, "onec": onec, "onef": onef,
        })
    return in_maps


def kernel(x, Wq, bq, Wk, bk, Wv, bv, Wo, bo):
    x = np.asarray(x, dtype=np.float32)
    Wq, Wk, Wv, Wo = (np.asarray(a, np.float32) for a in (Wq, Wk, Wv, Wo))
    bq, bk, bv, bo = (np.asarray(a, np.float32) for a in (bq, bk, bv, bo))
    in_maps = make_in_maps(x, Wq, bq, Wk, bk, Wv, bv, Wo, bo)
    nc = _get_nc()
    res = run_bass_kernel_spmd(nc, in_maps, core_ids=list(range(N_CORES)),
                               trace=False)
    out = np.empty((B, S, D), np.float32)
    for c in range(N_CORES):
        b, h = c // 2, c % 2
        out[b, h * SQ:(h + 1) * SQ] = res.results[c]["y"]
    return out


if __name__ == "__main__":
    rng = np.random.default_rng(0)
    x = rng.standard_normal((B, S, D), dtype=np.float32)
    s_in, s_r = 1.0 / np.sqrt(D), 1.0 / np.sqrt(R)
    mk = lambda sh, s: rng.uniform(-s, s, sh).astype(np.float32)
    out = kernel(x, mk((D, R), s_in), mk((R,), s_in), mk((D, R), s_in),
                 mk((R,), s_in), mk((D, R), s_in), mk((R,), s_in),
                 mk((R, D), s_r), mk((D,), s_r))
    print("ran ok", out.shape, out[0, 0, :4])

